# revision 53
# baseline (speedup 1.0000x reference)
"""Multi-head attention (B=1, S=4096, D=768, 12 heads) on 8 trn2 cores.

Sharding: tensor-parallel by heads, balanced with sequence splits.
Core c owns: head A = c (all 4096 query rows) and head B = 8 + c//2
(query-row half c%2).  Each core computes q/k/v for its two heads, full
S x S attention for its share, and its heads' partial contribution to
the output projection (row-parallel split of w_proj).  The host divides
the per-core unnormalized projection partials by the shipped softmax
denominators, sums them, and adds the bias.

Device layout: d-on-partitions ("transposed") everywhere.  Scores are
computed as S^T[t, s] = K^T.T @ Q^T per 128-key chunk into one PSUM
bank per unit, so ScalarE (L unit, table exp) and the DVE (R unit,
bit-trick exp) read disjoint banks concurrently.  The AV matmul packs
[V | ones] (L) / [ones | V] (R) as the stationary operand so O^T and
the denominator accumulate in one pass, with L's O^T on partitions
0-63 and R's on 64-127 - which also lets the w_proj chunks of both
units run as row-group-paired concurrent matmuls.

The attention loop runs "sweeps" that process two units at once so the
K=64 score matmuls run pairwise concurrent on the PE array; the next
tile's scores are emitted ahead of this tile's AV matmuls (the PE
queue is strictly in-order).  Head A's second half pairs with itself
via a partition-shifted duplicate of Q^T/K^T.  q/k/v generation is
woven into sweeps 0-2 chunk-paced with the input DMA (all bulk input
on one queue, in need-order); V is computed directly in [keys, hd]
layout on the PE (no DMA transposes).  Dummy matmuls bridge the final
drain so HAM keeps the PE at full clock through the projection tail.
All matmuls are bf16 with fp32 PSUM accumulation.
"""

import numpy as np
import ml_dtypes

import concourse.bass as bass
import concourse.mybir as mybir
import concourse.tile as tile
from concourse import bacc
from concourse.bass_utils import run_bass_kernel_spmd

BF16 = mybir.dt.bfloat16
F32 = mybir.dt.float32
ts = bass.ts
ds = bass.ds

S = 4096
D = 768
NH = 12
HD = 64
NCORES = 8
SU = 2048          # rows per unit
PO = D // 128      # 6 e-chunks
NT = S // 128      # 32 key chunks
SCALE = HD ** -0.5
SPL = 672          # exp split point: ScalarE does [0:SPL], DVE the rest

_CACHE: dict = {}

# --- custom DVE exp op: out_uint16 = bf16 bits of 2^((x - 64)/128) ---------
# Magic-constant round to the 128-grid + quadratic mantissa correction,
# emitted through the fp32->uint16 value cast.  The -64 window shift (a
# global 2^-0.5 factor on all exp values) cancels in the softmax
# normalization; the ScalarE branch matches it via the activation bias.
EXP_M = 1.5 * 2**30
EXP_Q0 = 16180.991964579287
EXP_Q1 = 0.9950478871994926
EXP_Q2 = 0.0026875086476569427
EXP_SCALE = float(np.log(2) / 128.0)
EXP_BIAS = float(-np.log(2) / 2.0)
LOG2E_128 = float(128.0 / np.log(2))


def _expb_ref(in0, in1, s0, s1, imm2):
    f32 = np.float32
    a = (in0.astype(f32) + f32(s0)).astype(f32)
    u = (a - f32(s0)).astype(f32)
    z = (in0.astype(f32) - u).astype(f32)
    m2 = (((z * f32(s1)).astype(f32) + f32(imm2)).astype(f32) * z).astype(f32)
    return ((u + m2).astype(f32) + in1.astype(f32)).astype(f32)


def _expb_op():
    from concourse import dve_ops
    from concourse.dve_spec import Spec, Src0, C0, C1, C2, C3, lower, _spill_c3_to_src1
    from concourse.dve_uop import DveOpSpec

    for op in dve_ops.OPS:
        if op.name == "EXPB_ANT":
            return op
    a = Src0 + C0
    u = a - C0
    z = Src0 - u
    m2 = (z * C1 + C2) * z
    body = _spill_c3_to_src1((u + m2) + C3)
    spec = Spec(body=body, reference=_expb_ref)
    row = dve_ops._CUSTOM_DVE_ROW_BASE + len(dve_ops.OPS)
    dve_ops._SUB_OPCODE_FOR_NAME["EXPB_ANT"] = row
    shas = {}
    for ver in ("v3", "v4"):
        try:
            uops = lower(spec, ver=ver)
            shas[ver] = DveOpSpec(
                name="EXPB_ANT", opcode=row, uops=uops, rd1_en=True
            ).sha(ver)
        except Exception:
            pass
    op = dve_ops.DveOp("EXPB_ANT", spec, subdim=False, uops_sha=shas)
    dve_ops.OPS.append(op)
    dve_ops.CUSTOM_DVE_SPECS["EXPB_ANT"] = spec
    return op


def _emit(nc):
    xT = nc.dram_tensor("xT", [D, S], BF16, kind="ExternalInput")
    xq = nc.dram_tensor("xq", [D, SU], BF16, kind="ExternalInput")
    wq = nc.dram_tensor("wq", [D, 128], BF16, kind="ExternalInput")
    wk = nc.dram_tensor("wk", [D, 128], BF16, kind="ExternalInput")
    wv = nc.dram_tensor("wv", [D, 128], BF16, kind="ExternalInput")
    wp = nc.dram_tensor("wp", [128, 2, D], BF16, kind="ExternalInput")
    yTa = nc.dram_tensor("yTa", [D, S], F32, kind="ExternalOutput")
    yTb = nc.dram_tensor("yTb", [D, SU], F32, kind="ExternalOutput")
    da = nc.dram_tensor("da", [1, S], F32, kind="ExternalOutput")
    db = nc.dram_tensor("db", [1, SU], F32, kind="ExternalOutput")

    xT_v = xT.ap().rearrange("(po pi) s -> pi po s", pi=128)
    xq_v = xq.ap().rearrange("(po pi) s -> pi po s", pi=128)
    wq_v = wq.ap().rearrange("(po pi) o -> pi po o", pi=128)
    wk_v = wk.ap().rearrange("(po pi) o -> pi po o", pi=128)
    wv_v = wv.ap().rearrange("(po pi) o -> pi po o", pi=128)
    yTa_v = yTa.ap().rearrange("(po pi) s -> pi po s", pi=128)
    yTb_v = yTb.ap().rearrange("(po pi) s -> pi po s", pi=128)

    with tile.TileContext(nc) as tc:
        with (
            tc.tile_pool(name="persist", bufs=1) as pp,
            tc.tile_pool(name="pt", bufs=4) as ptp,
            tc.tile_pool(name="ot", bufs=3) as otp,
            tc.tile_pool(name="rb", bufs=3) as rbp,
            tc.tile_pool(name="yt", bufs=8) as ytp,
            tc.tile_pool(name="sc", bufs=4, space="PSUM") as scp,
            tc.tile_pool(name="av", bufs=4, space="PSUM") as avp,
        ):
            # ---------------- persistent SBUF tensors ----------------
            xT_sb = pp.tile([128, PO, S], BF16)
            xq_sb = pp.tile([128, PO, SU], BF16)
            wq_sb = pp.tile([128, PO, 128], BF16)
            wk_sb = pp.tile([128, PO, 128], BF16)
            wv_sb = pp.tile([128, PO, 128], BF16)
            wp_sb = pp.tile([128, 2, D], BF16)   # rows 64:128 duplicate 0:64 (proj row-pairing)
            QT_sb = pp.tile([128, S], BF16)      # 0:64 A (full S); 64:128 B (cols 0:SU) + A-dup (cols 3072:4096)
            KT_sb = pp.tile([128, S], BF16)      # 0:64 A, 64:128 B
            KT2_sb = pp.tile([128, S], BF16)     # 64:128 = copy of A rows (for self-pairing)
            # L-side AVs use [V|ones] (O^T rows 0:64), R-side AVs [ones|V]
            # (O^T rows 64:128) so an L/R proj pair spans the full 128
            # partitions for row-group-paired matmuls
            V_sb = pp.tile([128, NT, 384], BF16)  # [V_A|ones | ones|V_B | ones|V_A]

            # bulk input split over both DMA queues, <=10 DMAs each so the
            # framework never has to recycle completion semaphores mid-stream
            # xT and xq share the sync queue so they are served strictly in
            # need-order (two queues would compete for HBM and delay xT0);
            # only the small weights ride gpsimd
            nc.gpsimd.dma_start(wq_sb[:], wq_v)
            nc.gpsimd.dma_start(wv_sb[:], wv_v)
            nc.gpsimd.dma_start(wk_sb[:], wk_v)
            for n in range(8):
                nc.sync.dma_start(xT_sb[:, :, ts(n, 512)], xT_v[:, :, ts(n, 512)])
                if n < 4:
                    nc.sync.dma_start(xq_sb[:, :, ts(n, 512)], xq_v[:, :, ts(n, 512)])
            nc.gpsimd.dma_start(wp_sb[:], wp.ap())
            q0_sb = pp.tile([128, 1], F32)
            bias_sb = pp.tile([128, 1], F32)
            # HAM pre-warm: dense matmuls while input DMAs stream, so the PE
            # clock is at 2.4 GHz when real work starts.  warm_sb's memset goes
            # first on the vector queue; the big V_sb ones-memsets ride gpsimd
            # so they can't delay it.
            warm_sb = pp.tile([128, 128], BF16)
            nc.vector.memset(warm_sb[:], 0.0)
            nc.vector.memset(q0_sb[:], EXP_Q0)
            nc.vector.memset(bias_sb[:], EXP_BIAS)
            nc.vector.memset(V_sb[:, :, 64:128], 1.0)
            nc.vector.memset(V_sb[:, :, 128:192], 1.0)
            nc.vector.memset(V_sb[:, :, 256:320], 1.0)
            warm_ps = avp.tile([128, 512], F32, tag="av", name="warm_ps")
            for i in range(64):
                nc.tensor.matmul(
                    warm_ps[:, 0:128], lhsT=warm_sb[:], rhs=warm_sb[:],
                    start=(i == 0), stop=(i == 63),
                )
            expb = _expb_op()

            # ---------------- q/k/v projections (woven into sweep 0) -------
            pfx_alt = [0]

            def pfx_copy(dst, srcv):
                pfx_alt[0] ^= 1
                if pfx_alt[0]:
                    nc.vector.tensor_copy(dst, srcv)
                else:
                    nc.scalar.copy(dst, srcv)

            def emit_qt_block(n):
                qa_ps = scp.tile([128, 512], F32, tag="sc", name="qa_ps")
                qb_ps = avp.tile([128, 512], F32, tag="av", name="qb_ps") if n < 4 else None
                # qa first as a full group: it only needs xT, while qb waits on
                # the later-arriving xq - interleaving would block qa (PE queue
                # is in-order)
                for po in range(PO):
                    nc.tensor.matmul(
                        qa_ps[0:64, 0:512],
                        lhsT=wq_sb[:, po, 0:64],
                        rhs=xT_sb[:, po, ts(n, 512)],
                        start=(po == 0),
                        stop=(po == PO - 1),
                    )
                for po in range(PO):
                    if qb_ps is not None:
                        nc.tensor.matmul(
                            qb_ps[64:128, :],
                            lhsT=wq_sb[:, po, 64:128],
                            rhs=xq_sb[:, po, ts(n, 512)],
                            start=(po == 0),
                            stop=(po == PO - 1),
                        )
                pfx_copy(QT_sb[0:64, ts(n, 512)], qa_ps[0:64, 0:512])
                if qb_ps is not None:
                    pfx_copy(QT_sb[64:128, ts(n, 512)], qb_ps[64:128, :])

            def emit_qt_pair_high(nlo, nhi):
                # head-A q for block nlo -> QT rows 0:64, block nhi -> rows
                # 64:128 (the self-pair dup region reads only rows 64:128 for
                # cols 3072:4096, so compute it there directly).  The two
                # matmuls col-group-pair and run concurrently.
                qa_ps = scp.tile([128, 512], F32, tag="sc", name="qa_ps")
                for po in range(PO):
                    nc.tensor.matmul(
                        qa_ps[0:64, :],
                        lhsT=wq_sb[:, po, 0:64],
                        rhs=xT_sb[:, po, ts(nlo, 512)],
                        start=(po == 0),
                        stop=(po == PO - 1),
                    )
                    nc.tensor.matmul(
                        qa_ps[64:128, :],
                        lhsT=wq_sb[:, po, 0:64],
                        rhs=xT_sb[:, po, ts(nhi, 512)],
                        start=(po == 0),
                        stop=(po == PO - 1),
                    )
                pfx_copy(QT_sb[0:64, ts(nlo, 512)], qa_ps[0:64, :])
                pfx_copy(QT_sb[64:128, ts(nhi, 512)], qa_ps[64:128, :])

            def emit_kt_block(n):
                k_ps = avp.tile([128, 512], F32, tag="av", name="k_ps")
                for po in range(PO):
                    nc.tensor.matmul(
                        k_ps[:, 0:512],
                        lhsT=wk_sb[:, po, :],
                        rhs=xT_sb[:, po, ts(n, 512)],
                        start=(po == 0),
                        stop=(po == PO - 1),
                    )
                pfx_copy(KT_sb[:, ts(n, 512)], k_ps[:, 0:512])

            def emit_vt_block(n):
                # V computed directly in [keys, hd] layout: 4 key-chunks per
                # block, each a 6-step accumulation with xT stationary.  Costs
                # more PE than the V^T orientation but avoids the DMA-transpose
                # path entirely (which serializes the DMA rings).
                v_ps = avp.tile([128, 512], F32, tag="av", name="v_ps")
                for k in range(4):
                    for po in range(PO):
                        nc.tensor.matmul(
                            v_ps[:, ts(k, 128)],
                            lhsT=xT_sb[:, po, ds(n * 512 + k * 128, 128)],
                            rhs=wv_sb[:, po, 0:128],
                            start=(po == 0),
                            stop=(po == PO - 1),
                        )
                src2 = v_ps[:].rearrange("p (k c) -> p k c", k=4)
                pfx_copy(V_sb[:, 4 * n : 4 * n + 4, 0:64], src2[:, :, 0:64])
                pfx_copy(V_sb[:, 4 * n : 4 * n + 4, 192:256], src2[:, :, 64:128])
                pfx_copy(V_sb[:, 4 * n : 4 * n + 4, 320:384], src2[:, :, 0:64])

            # ---------------- attention sweeps ----------------
            da_v = da.ap()
            db_v = db.ap()
            # sides: (kt_tile, slot, qt_abs_col, vbase)
            # per-sweep spec: (otT, otcol, [(wpi, ydst, ycol) x2], [(den_dst, dcol) x2])
            ot03 = otp.tile([128, 2048], BF16, tag="ot", name="ot03")
            ot45 = otp.tile([128, 1024], BF16, tag="ot", name="ot45")
            sweeps = []
            pspecs = []
            for sb in range(4):
                sweeps.append(
                    (
                        (KT_sb, 0, sb * 512, 0),      # u0: head A q 0:2048
                        (KT_sb, 64, sb * 512, 128),   # u2: head B
                    )
                )
                pspecs.append(
                    (
                        ot03,
                        sb * 512,
                        [(0, yTa_v, sb * 512), (1, yTb_v, sb * 512)],
                        [(da_v, sb * 512), (db_v, sb * 512)],
                    )
                )
            for sb in range(2):
                sweeps.append(
                    (
                        (KT_sb, 0, SU + sb * 512, 0),            # u1: head A q 2048+
                        (KT2_sb, 64, SU + 1024 + sb * 512, 256),  # u1 self-pair
                    )
                )
                pspecs.append(
                    (
                        ot45,
                        sb * 512,
                        [(0, yTa_v, SU + sb * 512), (0, yTa_v, SU + 1024 + sb * 512)],
                        [(da_v, SU + sb * 512), (da_v, SU + 1024 + sb * 512)],
                    )
                )

            pump_clock = [0]
            cp_q = []
            tail_mode = [False]
            cp_alt = [0]

            def emit_proj_copy_q(fn):
                cp_q.append((pump_clock[0], fn))

            def flush_cp(age_min=0):
                while cp_q and pump_clock[0] - cp_q[0][0] >= age_min:
                    cp_q.pop(0)[1]()

            proj_q = []
            mul_q = []

            def emit_proj_copy(pj, ydst, oe, dcol):
                yt = ytp.tile([128, 512], F32, tag="yt", name="yt")
                # alternate the PSUM->SBUF copies across ScalarE and DVE
                cp_alt[0] ^= 1
                if not cp_alt[0]:
                    nc.vector.tensor_copy(yt[:], pj[:])
                else:
                    nc.scalar.copy(yt[:], pj[:])
                if cp_alt[0]:
                    nc.sync.dma_start(ydst[:, oe, ds(dcol, 512)], yt[:])
                else:
                    nc.gpsimd.dma_start(ydst[:, oe, ds(dcol, 512)], yt[:])

            def emit_proj_pair(oe, otT, otcol, pA, pB):
                # L chunk on PE rows 0:64, R chunk on rows 64:128 - concurrent
                wpiA, ydstA, ycolA = pA
                wpiB, ydstB, ycolB = pB
                pjA = avp.tile([128, 512], F32, tag="av", name="pj")
                pjB = avp.tile([128, 512], F32, tag="av", name="pj")
                nc.tensor.matmul(
                    pjA[:],
                    lhsT=wp_sb[0:64, wpiA, ts(oe, 128)],
                    rhs=otT[0:64, ds(otcol, 512)],
                    start=True,
                    stop=True,
                )
                nc.tensor.matmul(
                    pjB[:],
                    lhsT=wp_sb[64:128, wpiB, ts(oe, 128)],
                    rhs=otT[64:128, ds(otcol, 512)],
                    start=True,
                    stop=True,
                )
                emit_proj_copy_q(lambda: emit_proj_copy(pjA, ydstA, oe, ycolA))
                emit_proj_copy_q(lambda: emit_proj_copy(pjB, ydstB, oe, ycolB))

            def pump(t):
                pump_clock[0] += 1
                flush_cp(age_min=1)
                if mul_q:
                    if t >= 1:
                        mul_q.pop(0)()
                    return
                # every other tile: a pair's two PSUM banks are recycled by
                # copies that need ~1.4us; back-to-back pairs would stall the PE
                if t >= 3 and t % 2 == 1 and proj_q:
                    proj_q.pop(0)()

            for si, (Lside, Rside) in enumerate(sweeps):
                if si == 1:
                    # duplicate for head-A self-pairing (KT complete after sweep 0)
                    nc.gpsimd.dma_start(KT2_sb[64:128, :], KT_sb[0:64, :])
                avLR = [None, None]

                def emit_scores(t):
                    # one PSUM bank per unit so ScalarE (L) and DVE (R) exp can
                    # read their banks concurrently
                    pair = []
                    for (ktt, slot, qcol, _vb) in (Lside, Rside):
                        sc = scp.tile([128, 512], F32, tag="sc", name="sc")
                        nc.tensor.matmul(
                            sc[:],
                            lhsT=ktt[slot : slot + 64, ts(t, 128)],
                            rhs=QT_sb[slot : slot + 64, ds(qcol, 512)],
                            start=True,
                            stop=True,
                        )
                        pair.append(sc)
                    return pair

                sc_cur = None
                for t in range(NT):
                    if si == 0:
                        n = t // 4
                        if t == 0:
                            # kt/vt first: they depend only on xT0; qt's qb
                            # half waits for xq0, the last input to arrive
                            emit_kt_block(0)
                            emit_vt_block(0)
                            emit_qt_block(0)
                        elif t % 4 == 1 and n < 7:
                            emit_kt_block(n + 1)
                            emit_vt_block(n + 1)
                        elif t % 4 == 3 and n < 3:
                            emit_qt_block(n + 1)
                    elif si in (1, 2) and t == 1:
                        # qt blocks 4-7 are only needed by sweeps 4-5; 6-7 go
                        # straight to QT rows 64:128 (paired with 4-5)
                        emit_qt_pair_high(si + 3, si + 5)
                    if sc_cur is None:
                        sc_cur = emit_scores(0)
                    pt = ptp.tile([128, 1024], BF16, tag="pt", name="pt")
                    nc.scalar.activation(
                        pt[:, 0:512],
                        sc_cur[0][:],
                        mybir.ActivationFunctionType.Exp,
                        bias=bias_sb[:],
                        scale=EXP_SCALE,
                    )
                    nc.vector._custom_dve(
                        expb,
                        out=pt[:, 512:1024].bitcast(mybir.dt.uint16),
                        in0=sc_cur[1][:],
                        in1=q0_sb[:],
                        s0=EXP_M,
                        s1=EXP_Q2,
                        imm2=EXP_Q1,
                    )
                    # lookahead: next tile's scores go to the PE queue ahead of
                    # this tile's AV matmuls so the PE never waits on exp
                    sc_next = emit_scores(t + 1) if t < NT - 1 else None
                    pump(t)
                    if t == 0:
                        # allocate the accumulators only now so the sweep-0 qkv
                        # transients aren't starved of avp pool slots
                        avLR[0] = avp.tile([128, 512], F32, tag="av", name="avL")
                        avLR[1] = avp.tile([128, 512], F32, tag="av", name="avR")
                    for (_ktt, _slot, _qcol, vbase), (av, half) in (
                        (Lside, (avLR[0], 0)),
                        (Rside, (avLR[1], 1)),
                    ):
                        nc.tensor.matmul(
                            av,
                            lhsT=V_sb[:, t, vbase : vbase + 128],
                            rhs=pt[:, ts(half, 512)],
                            start=(t == 0),
                            stop=(t == NT - 1),
                        )
                    sc_cur = sc_next
                avL, avR = avLR
                flush_cp()
                if si == len(sweeps) - 1:
                    # keep the PE busy across the final recip/mul drain so HAM
                    # stays at full clock for the proj tail
                    warm2 = scp.tile([128, 512], F32, tag="sc", name="warm2")
                    for i in range(20):
                        nc.tensor.matmul(
                            warm2[:], lhsT=warm_sb[:], rhs=QT_sb[:, 0:512],
                            start=(i == 0), stop=(i == 19),
                        )
                # drain: no on-device normalization - ship unnormalized O^T
                # plus the softmax denominators; the host divides.  Deferred
                # into the next sweep's first tiles to keep the DVE queue
                # clear for its exp ops.
                otT, otcol, pAB, dAB = pspecs[si]

                def emit_drain_side(av, half, otT=otT, otcol=otcol, dAB=dAB):
                    den = rbp.tile([128, 512], F32, tag="rhi", name="den")
                    dd, dcol = dAB[half]
                    if half == 0:
                        # L: O^T rows 0:64, denominator (ones rows) 64:128
                        nc.vector.tensor_copy(otT[0:64, ds(otcol, 512)], av[0:64, :])
                        nc.scalar.copy(den[64:65, :], av[64:65, :])
                        nc.sync.dma_start(dd[0:1, ds(dcol, 512)], den[64:65, :])
                    else:
                        # R: denominator rows 0:64, O^T rows 64:128; this O^T
                        # copy rides ScalarE so each engine absorbs one 690ns
                        # copy instead of the DVE absorbing both
                        nc.scalar.copy(otT[64:128, ds(otcol, 512)], av[64:128, :])
                        nc.vector.tensor_copy(den[0:1, :], av[0:1, :])
                        nc.sync.dma_start(dd[0:1, ds(dcol, 512)], den[0:1, :])

                mul_q.append(lambda av=avL: emit_drain_side(av, 0))
                mul_q.append(lambda av=avR: emit_drain_side(av, 1))
                for oe in range(PO):
                    proj_q.append(
                        lambda oe=oe, otT=otT, otcol=otcol, pAB=pAB: emit_proj_pair(
                            oe, otT, otcol, pAB[0], pAB[1]
                        )
                    )
            tail_mode[0] = True
            while mul_q or proj_q or cp_q:
                if mul_q:
                    # drain copies must be emitted before the proj matmuls
                    # that read them (readers emitted first are not ordered)
                    mul_q.pop(0)()
                elif proj_q:
                    proj_q.pop(0)()
                flush_cp()

    nc.compile()
    return nc


def _build():
    if "nc" not in _CACHE:
        nc = bacc.Bacc(None, target_bir_lowering=False, debug=False)
        _CACHE["nc"] = _emit(nc)
    return _CACHE["nc"]


def _prep_inputs(x, w_qkv, w_proj):
    bf = ml_dtypes.bfloat16
    xs = np.ascontiguousarray(x.reshape(S, D).T).astype(bf)  # [D, S]
    in_maps = []
    for c in range(NCORES):
        ha = c
        hb = 8 + c // 2
        bh = c % 2
        rows_q = lambda h: w_qkv[h * HD : (h + 1) * HD, :]
        rows_k = lambda h: w_qkv[D + h * HD : D + (h + 1) * HD, :]
        rows_v = lambda h: w_qkv[2 * D + h * HD : 2 * D + (h + 1) * HD, :]
        qs = SCALE * LOG2E_128
        wq_c = np.concatenate([rows_q(ha) * qs, rows_q(hb) * qs], 0).T
        wk_c = np.concatenate([rows_k(ha), rows_k(hb)], 0).T
        wv_c = np.concatenate([rows_v(ha), rows_v(hb)], 0).T
        wp_c = np.stack(
            [w_proj[:, ha * HD : (ha + 1) * HD].T, w_proj[:, hb * HD : (hb + 1) * HD].T],
            axis=1,
        )  # [64, 2, D]
        wp_c = np.concatenate([wp_c, wp_c], axis=0)  # [128, 2, D] for row-pairing
        in_maps.append(
            {
                "xT": xs,
                "xq": np.ascontiguousarray(xs[:, bh * SU : (bh + 1) * SU]),
                "wq": np.ascontiguousarray(wq_c).astype(bf),
                "wk": np.ascontiguousarray(wk_c).astype(bf),
                "wv": np.ascontiguousarray(wv_c).astype(bf),
                "wp": np.ascontiguousarray(wp_c).astype(bf),
            }
        )
    return in_maps


def _combine(results, b_proj):
    yT = np.zeros((D, S), np.float32)
    for c in range(NCORES):
        r = results[c]
        yT += r["yTa"] / r["da"][0][None, :]
        bh = c % 2
        yT[:, bh * SU : (bh + 1) * SU] += r["yTb"] / r["db"][0][None, :]
    y = yT.T + b_proj.astype(np.float32)[None, :]
    return y.reshape(1, 64, 64, D).astype(np.float32)


def kernel(x, w_qkv, w_proj, b_proj, _trace=False, _trace_kwargs=None):
    x = np.asarray(x, np.float32)
    w_qkv = np.asarray(w_qkv, np.float32)
    w_proj = np.asarray(w_proj, np.float32)
    b_proj = np.asarray(b_proj, np.float32)

    nc = _build()
    in_maps = _prep_inputs(x, w_qkv, w_proj)
    res = run_bass_kernel_spmd(
        nc, in_maps, core_ids=list(range(NCORES)), trace=_trace,
        **(_trace_kwargs or {}),
    )
    out = _combine(res.results, b_proj)
    if _trace:
        return out, res
    return out

